# revision 7
# baseline (speedup 1.0000x reference)
"""AttnBlock++ (GroupNorm -> q/k/v 1x1 -> full LxL attention -> proj -> residual)
on 8 Trainium2 NeuronCores, data-parallel over batch (one batch element per core).

Strategy: all heavy matmuls in fp8 (e4m3) DoubleRow mode (0.5 cyc/row, K=256 per
instruction = 4x the f32r rate), with hi-lo fp8 pairs (value = hi + lo, both
e4m3) recovering ~bf16-grade accuracy at half the bf16 PE cost:

  - bilinear trick: scores = xn^T (Wk' Wq'^T) xn, so M := 16*(Wk' Wq'^T) is
    precomputed on the host (gamma folded into W rows) and only ONE projection
    t = M^T(u - D) is needed on-device instead of q and k.
  - GroupNorm: bn_stats/bn_aggr + group-combine matmul give per-channel
    rstd (A) and mean*rstd (D).  u = x*A is quantized hi-lo; the -D shift
    goes into the t projection (moving side) where its per-query component
    cancels in softmax; the per-key component's D^T M D / bias terms cancel
    or fold into biases.  q/k biases are zero for this model family (host-
    asserted); v bias + gn beta fold into b3 on the host.
  - scores (3 hi-lo cross terms, dropping lo*lo), probs = exp(s/256 - 1) in
    fp8e5 (e5m2: its ~21 ln-unit range covers the empirical score span
    [-10.7, 11.2]; quantization error ~7% rms on probs is within budget).
  - exp split across engines: most pairs on ScalarE (native Exp -> e5m2),
    a few via DVE Schraudolph (bits = round(s*m + c) as int16 -> bf16 bit
    pattern, Pool converts bf16 -> e5m2).
  - denominator Z via an all-16s fp8 DoubleRow matmul (rps = 16Z broadcast
    across partitions), so att = pv * reciprocal(rps) lands exactly at h
    (w3 is host-quantized UNSCALED so out = w3^T h needs no rescale).
  - pv: v hi-lo x probs DoubleRow; out-proj: w3 hi-lo x att hi-lo (3 terms);
    final drain fuses +b3' + x residual in one scalar_tensor_tensor.

Engines: PE ~45us of matmul; exp + drains + quantize passes balanced across
ScalarE / VectorE / Pool.  Expected ~2x vs the f32r baseline (95us).
Max rel err ~1.4e-2 predicted by the numpy pipeline model (gate 2e-2).
"""

import numpy as np
import ml_dtypes

import concourse.bacc as bacc
import concourse.mybir as mybir
import concourse.tile as tile
from concourse.bass_utils import run_bass_kernel_spmd

f32 = mybir.dt.float32
f32r = mybir.dt.float32r
e4 = mybir.dt.float8e4
e5 = mybir.dt.float8e5
i16 = mybir.dt.int16
bf16 = mybir.dt.bfloat16
E4 = ml_dtypes.float8_e4m3
E5 = ml_dtypes.float8_e5m2

AF = mybir.ActivationFunctionType
DR = mybir.MatmulPerfMode.DoubleRow
ALU = mybir.AluOpType

B, C, L = 8, 256, 2048
G = 32
EPS = 1e-6
CT = C // 128             # 2 channel tiles
NCH = L // 512            # 4 query chunks
KB = L // 128             # 16 key blocks
NPAIR = KB // 2           # 8 key-block pairs per chunk
WS = 16.0                 # host weight pre-scale on M and w2
SCALE_EXP = (C ** -0.5) / WS   # probs = exp(s_raw * SCALE_EXP - T); only t carries WS
T_SHIFT = 1.0
LOG2E = 1.4426950408889634
SCH_C = -6.0              # schraudolph constant (HW f32->i16 rounds)
ONESV = 16.0              # denominator ones value: rps = 16*Z

# exp path per global pair index (chunk*8 + pair): True = schraudolph (DVE+Pool)
SCH_PAIRS = {3, 7, 11, 15, 19, 23, 27, 31} if True else set()


def _build(nrep=1):
    nc = bacc.Bacc(trn_type="TRN2")

    x_d = nc.dram_tensor("x", (C, L), f32, kind="ExternalInput")
    mt_h_d = nc.dram_tensor("mt_h", (C, C), e4, kind="ExternalInput")
    mt_l_d = nc.dram_tensor("mt_l", (C, C), e4, kind="ExternalInput")
    w2_h_d = nc.dram_tensor("w2_h", (C, C), e4, kind="ExternalInput")
    w2_l_d = nc.dram_tensor("w2_l", (C, C), e4, kind="ExternalInput")
    w3_h_d = nc.dram_tensor("w3_h", (C, C), e4, kind="ExternalInput")
    w3_l_d = nc.dram_tensor("w3_l", (C, C), e4, kind="ExternalInput")
    b3p_d = nc.dram_tensor("b3p", (C,), f32, kind="ExternalInput")
    out_d = nc.dram_tensor("out", (C, L), f32, kind="ExternalOutput")

    # group averaging matrix: P[c',c] = 1/8 where same 8-channel group
    blob_np = ((np.arange(128)[:, None] // 8) == (np.arange(128)[None, :] // 8))
    blob_np = blob_np.astype(np.float32) / 8.0
    blob_d = nc.inline_tensor(blob_np, "gblob")
    ones_np = np.full((128, 256), ONESV, np.float32).astype(E4)
    ones_d = nc.inline_tensor(ones_np.view(np.uint8), "ones16")

    with tile.TileContext(nc) as tc:
        with tc.tile_pool(name="const", bufs=1) as cp, \
             tc.tile_pool(name="data", bufs=1) as dp, \
             tc.tile_pool(name="small", bufs=1) as sp, \
             tc.tile_pool(name="probs", bufs=6) as pp, \
             tc.tile_pool(name="attp", bufs=2) as ap_, \
             tc.tile_pool(name="fin", bufs=4) as fp_, \
             tc.tile_pool(name="ps", bufs=1, space="PSUM") as ps:

            # ---------- persistent tiles ----------
            xf = dp.tile([128, CT, L], f32, tag="xf", name="xf")
            uh = dp.tile([128, CT, L], e4, tag="uh", name="uh")
            ul = dp.tile([128, CT, L], e4, tag="ul", name="ul")
            th = dp.tile([128, CT, L], e4, tag="th", name="th")
            tl = dp.tile([128, CT, L], e4, tag="tl", name="tl")
            vh = dp.tile([128, KB, C], e4, tag="vh", name="vh")
            vl = dp.tile([128, KB, C], e4, tag="vl", name="vl")

            # ---------- input DMAs ----------
            for t in range(CT):
                for j in range(4):
                    nc.sync.dma_start(
                        out=xf[:, t, j * 512:(j + 1) * 512],
                        in_=x_d[t * 128:(t + 1) * 128, j * 512:(j + 1) * 512])

            wt = {}
            for nm, d in (("mt_h", mt_h_d), ("mt_l", mt_l_d),
                          ("w2_h", w2_h_d), ("w2_l", w2_l_d),
                          ("w3_h", w3_h_d), ("w3_l", w3_l_d)):
                tl_ = cp.tile([128, CT, C], e4, tag=nm, name=nm)
                for t in range(CT):
                    nc.gpsimd.dma_start(out=tl_[:, t, :],
                                        in_=d[t * 128:(t + 1) * 128, :])
                wt[nm] = tl_

            gblob = cp.tile([128, 128], f32, tag="gblob", name="gblob")
            nc.gpsimd.dma_start(out=gblob[:], in_=blob_d[:, :])
            ones16 = cp.tile([128, 2, 128], e4, tag="ones16", name="ones16")
            nc.gpsimd.dma_start(out=ones16[:], in_=ones_d[:, :].bitcast(e4))
            b3p_sb = sp.tile([128, CT], f32, tag="b3p", name="b3p")
            nc.gpsimd.dma_start(out=b3p_sb[:],
                                in_=b3p_d.rearrange("(t p) -> p t", t=CT))

            eps128 = sp.tile([128, 1], f32, tag="eps128", name="eps128")
            nc.vector.memset(eps128[:], EPS)
            negT = sp.tile([128, 1], f32, tag="negT", name="negT")
            nc.vector.memset(negT[:], -T_SHIFT)
            negone = sp.tile([128, 1], f32, tag="negone", name="negone")
            nc.vector.memset(negone[:], -1.0)

            for _rep in range(nrep):
              # ---------- GroupNorm statistics ----------
              mc_l = []
              for t in range(CT):
                  stats = sp.tile([128, 4, 6], f32, tag=f"stats{t}", name=f"stats{t}")
                  for j in range(4):
                      nc.vector.bn_stats(out=stats[:, j, :],
                                         in_=xf[:, t, j * 512:(j + 1) * 512])
                  s_ = sp.tile([128, 2], f32, tag=f"s{t}", name=f"s{t}")
                  mv = sp.tile([128, 2], f32, tag=f"mv{t}", name=f"mv{t}")
                  nc.vector.bn_aggr(out=mv[:], in_=stats[:])
                  nc.vector.tensor_copy(s_[:, 0:1], mv[:, 0:1])
                  nc.vector.scalar_tensor_tensor(
                      out=s_[:, 1:2], in0=mv[:, 0:1], scalar=mv[:, 0:1],
                      in1=mv[:, 1:2], op0=ALU.mult, op1=ALU.add)
                  gps = ps.tile([128, 2], f32, tag="fp", name="fp", bufs=1)
                  nc.tensor.matmul(gps[:], gblob[:], s_[:], start=True, stop=True)
                  me = sp.tile([128, 2], f32, tag=f"me{t}", name=f"me{t}")
                  nc.scalar.copy(me[:], gps[:])
                  mc_l.append(me)

              A_t = sp.tile([128, CT], f32, tag="A", name="A")
              D8 = sp.tile([128, CT, 1], e4, tag="D8", name="D8")
              for t in range(CT):
                  me = mc_l[t]
                  m_c = me[:, 0:1]
                  gvar = sp.tile([128, 1], f32, tag=f"gvar{t}", name=f"gvar{t}")
                  nc.vector.scalar_tensor_tensor(
                      out=gvar[:], in0=m_c, scalar=m_c, in1=me[:, 1:2],
                      op0=ALU.mult, op1=ALU.subtract)
                  rstd = sp.tile([128, 1], f32, tag=f"rstd{t}", name=f"rstd{t}")
                  nc.scalar.activation(out=rstd[:], in_=gvar[:], func=AF.Sqrt,
                                       bias=eps128[:], scale=-1.0)
                  nc.vector.reciprocal(rstd[:], rstd[:])
                  nc.vector.tensor_copy(A_t[:, t:t + 1], rstd[:])
                  # D = m * rstd, straight to fp8 column for matmul folds
                  nc.vector.tensor_mul(D8[:, t, :], m_c, rstd[:])

              # ---------- u = x * A, hi-lo fp8 ----------
              for t in range(CT):
                  for j in range(2):
                      sl = slice(j * 1024, (j + 1) * 1024)
                      nc.scalar.activation(out=uh[:, t, sl], in_=xf[:, t, sl],
                                           func=AF.Copy, bias=0.0,
                                           scale=A_t[:, t:t + 1])
                      nc.vector.scalar_tensor_tensor(
                          out=ul[:, t, sl], in0=xf[:, t, sl],
                          scalar=A_t[:, t:t + 1], in1=uh[:, t, sl],
                          op0=ALU.mult, op1=ALU.subtract)

              # ---------- bias-fold matmuls ----------
              # mdcol[ci] = -(M16 D)_ci  (t-drain bias);  uses hi+lo of MT
              mdps = ps.tile([128, CT], f32, tag="fp", name="fp", bufs=1)
              for t in range(CT):
                  nc.tensor.matmul(mdps[:, t:t + 1],
                                   wt["mt_h"][:, :, t * 128:(t + 1) * 128],
                                   D8[:], start=True, stop=False, perf_mode=DR)
                  nc.tensor.matmul(mdps[:, t:t + 1],
                                   wt["mt_l"][:, :, t * 128:(t + 1) * 128],
                                   D8[:], start=False, stop=True, perf_mode=DR)
              mdcol = sp.tile([128, CT], f32, tag="mdcol", name="mdcol")
              nc.scalar.activation(out=mdcol[:], in_=mdps[:], func=AF.Copy,
                                   bias=0.0, scale=-1.0)

              # cfix8 = fp8((w2_16^T D)/16) column; b3col = b3p - w3^T cfix*16/16
              cfps = ps.tile([128, CT], f32, tag="fp", name="fp", bufs=1)
              for t in range(CT):
                  nc.tensor.matmul(cfps[:, t:t + 1],
                                   wt["w2_h"][:, :, t * 128:(t + 1) * 128],
                                   D8[:], start=True, stop=False, perf_mode=DR)
                  nc.tensor.matmul(cfps[:, t:t + 1],
                                   wt["w2_l"][:, :, t * 128:(t + 1) * 128],
                                   D8[:], start=False, stop=True, perf_mode=DR)
              cfix8 = sp.tile([128, CT, 1], e4, tag="cfix8", name="cfix8")
              nc.scalar.activation(out=cfix8[:, :, 0], in_=cfps[:], func=AF.Copy,
                                   bias=0.0, scale=1.0 / WS)
              bfps = ps.tile([128, CT], f32, tag="fp", name="fp", bufs=1)
              for t in range(CT):
                  nc.tensor.matmul(bfps[:, t:t + 1],
                                   wt["w3_h"][:, :, t * 128:(t + 1) * 128],
                                   cfix8[:], start=True, stop=False, perf_mode=DR)
                  nc.tensor.matmul(bfps[:, t:t + 1],
                                   wt["w3_l"][:, :, t * 128:(t + 1) * 128],
                                   cfix8[:], start=False, stop=True, perf_mode=DR)
              b3col = sp.tile([128, CT], f32, tag="b3col", name="b3col")
              # b3col = b3p - bfps  (cfix8 already /16 undoes w2's x16)
              nc.vector.scalar_tensor_tensor(
                  out=b3col[:], in0=bfps[:], scalar=negone[:], in1=b3p_sb[:],
                  op0=ALU.mult, op1=ALU.add)

              # ---------- t = M16^T(u - D): projection + hi-lo drains ----------
              for dt in range(CT):
                  tsl = slice(dt * 128, (dt + 1) * 128)
                  for np_ in range(NCH // 2):
                      mm = ps.tile([128, 2, 512], f32, tag="sp", name="sp", bufs=2)
                      for h in range(2):
                          qs = slice((2 * np_ + h) * 512, (2 * np_ + h + 1) * 512)
                          nc.tensor.matmul(mm[:, h, :], wt["mt_h"][:, :, tsl],
                                           uh[:, :, qs], start=True, stop=False,
                                           perf_mode=DR)
                          nc.tensor.matmul(mm[:, h, :], wt["mt_h"][:, :, tsl],
                                           ul[:, :, qs], start=False, stop=False,
                                           perf_mode=DR)
                          nc.tensor.matmul(mm[:, h, :], wt["mt_l"][:, :, tsl],
                                           uh[:, :, qs], start=False, stop=True,
                                           perf_mode=DR)
                      ws_ = slice(2 * np_ * 512, (2 * np_ + 2) * 512)
                      nc.scalar.activation(out=th[:, dt, ws_], in_=mm[:],
                                           func=AF.Identity,
                                           bias=mdcol[:, dt:dt + 1], scale=1.0)
                      nc.vector.scalar_tensor_tensor(
                          out=tl[:, dt, ws_], in0=mm[:],
                          scalar=mdcol[:, dt:dt + 1], in1=th[:, dt, ws_],
                          op0=ALU.add, op1=ALU.subtract)

              # ---------- v projection (transposed vT[i,c]), hi-lo ----------
              for pb in range(NPAIR):
                  mmt = ps.tile([128, 2, 512], f32, tag="sp", name="sp", bufs=2)
                  mm = mmt[:, :, 0:C]
                  for h in range(2):
                      ib = 2 * pb + h
                      isl = slice(ib * 128, (ib + 1) * 128)
                      nc.tensor.matmul(mm[:, h, :], uh[:, :, isl], wt["w2_h"][:],
                                       start=True, stop=False, perf_mode=DR)
                      nc.tensor.matmul(mm[:, h, :], ul[:, :, isl], wt["w2_h"][:],
                                       start=False, stop=False, perf_mode=DR)
                      nc.tensor.matmul(mm[:, h, :], uh[:, :, isl], wt["w2_l"][:],
                                       start=False, stop=True, perf_mode=DR)

                  vsl = slice(2 * pb, 2 * pb + 2)
                  nc.scalar.copy(vh[:, vsl, :], mm)
                  nc.vector.tensor_sub(vl[:, vsl, :], mm, vh[:, vsl, :])

              # ---------- attention ----------
              sp_tiles = {}

              def emit_scores(gp):
                  n, pb = divmod(gp, NPAIR)
                  qs = slice(n * 512, (n + 1) * 512)
                  spt = ps.tile([128, 2, 512], f32, tag="sp", name="sp", bufs=2)
                  for h in range(2):
                      ib = 2 * pb + h
                      isl = slice(ib * 128, (ib + 1) * 128)
                      nc.tensor.matmul(spt[:, h, :], uh[:, :, isl], th[:, :, qs],
                                       start=True, stop=False, perf_mode=DR)
                      nc.tensor.matmul(spt[:, h, :], uh[:, :, isl], tl[:, :, qs],
                                       start=False, stop=False, perf_mode=DR)
                      nc.tensor.matmul(spt[:, h, :], ul[:, :, isl], th[:, :, qs],
                                       start=False, stop=True, perf_mode=DR)
                  sp_tiles[gp] = spt

              emit_scores(0)
              emit_scores(1)
              for n in range(NCH):
                  qs = slice(n * 512, (n + 1) * 512)
                  pv = [ps.tile([128, 512], f32, tag=f"pv{t}", name=f"pv{t}")
                        for t in range(CT)]
                  rps = ps.tile([128, 512], f32, tag="rr", name="rr")
                  for pb in range(NPAIR):
                      gp = n * NPAIR + pb
                      spt = sp_tiles.pop(gp)
                      pr = pp.tile([128, 2, 512], e5, tag="pr", name="pr")
                      if gp in SCH_PAIRS:
                          bits = pp.tile([128, 2, 512], i16, tag="bits",
                                         name="bits", bufs=2)
                          nc.vector.tensor_scalar(
                              out=bits[:], in0=spt[:],
                              scalar1=SCALE_EXP * 128.0 * LOG2E,
                              scalar2=127.0 * 128 - T_SHIFT * 128.0 * LOG2E + SCH_C,
                              op0=ALU.mult, op1=ALU.add)
                          nc.gpsimd.tensor_copy(pr[:], bits[:].bitcast(bf16))
                      else:
                          nc.scalar.activation(out=pr[:], in_=spt[:], func=AF.Exp,
                                               bias=negT[:], scale=SCALE_EXP)
                      if gp + 2 < NCH * NPAIR:
                          emit_scores(gp + 2)
                      first, last = pb == 0, pb == NPAIR - 1
                      nc.tensor.matmul(rps[:], ones16[:], pr[:],
                                       start=first, stop=last, perf_mode=DR)
                      vsl = slice(2 * pb, 2 * pb + 2)
                      for t in range(CT):
                          tsl = slice(t * 128, (t + 1) * 128)
                          nc.tensor.matmul(pv[t][:], vh[:, vsl, tsl], pr[:],
                                           start=first, stop=False, perf_mode=DR)
                          nc.tensor.matmul(pv[t][:], vl[:, vsl, tsl], pr[:],
                                           start=False, stop=last, perf_mode=DR)

                  rinv = fp_.tile([128, 512], f32, tag="rinv", name="rinv")
                  nc.vector.reciprocal(rinv[:], rps[:])
                  attb = ap_.tile([128, CT, 512], bf16, tag="attb", name="attb")
                  ah8 = ap_.tile([128, CT, 512], e4, tag="ah8", name="ah8")
                  al8 = ap_.tile([128, CT, 512], e4, tag="al8", name="al8")
                  for t in range(CT):
                      nc.vector.tensor_mul(attb[:, t, :], pv[t][:], rinv[:])
                      nc.gpsimd.tensor_copy(ah8[:, t, :], attb[:, t, :])
                      nc.gpsimd.tensor_sub(al8[:, t, :], attb[:, t, :],
                                           ah8[:, t, :])

                  for t in range(CT):
                      tsl = slice(t * 128, (t + 1) * 128)
                      mm = ps.tile([128, 512], f32, tag="fp", name="fp", bufs=1)
                      nc.tensor.matmul(mm[:], wt["w3_h"][:, :, tsl], ah8[:],
                                       start=True, stop=False, perf_mode=DR)
                      nc.tensor.matmul(mm[:], wt["w3_h"][:, :, tsl], al8[:],
                                       start=False, stop=False, perf_mode=DR)
                      nc.tensor.matmul(mm[:], wt["w3_l"][:, :, tsl], ah8[:],
                                       start=False, stop=True, perf_mode=DR)
                      ob = fp_.tile([128, 512], f32, tag="outb", name="outb")
                      nc.vector.scalar_tensor_tensor(
                          out=ob[:], in0=mm[:], scalar=b3col[:, t:t + 1],
                          in1=xf[:, t, qs], op0=ALU.add, op1=ALU.add)
                      nc.sync.dma_start(out=out_d[t * 128:(t + 1) * 128, qs],
                                        in_=ob[:])

    nc.compile()
    return nc


_NC_CACHE = {}


def _get_nc(nrep=1):
    if nrep not in _NC_CACHE:
        _NC_CACHE[nrep] = _build(nrep)
    return _NC_CACHE[nrep]


def _host_prep(inputs):
    """Fold gamma/beta/biases and pre-quantize weights (layout/dtype prep)."""
    gam = np.asarray(inputs["gn_gamma"], np.float32)
    bet = np.asarray(inputs["gn_beta"], np.float32)
    w = [np.asarray(inputs[f"w{i}"], np.float32) for i in range(4)]
    b = [np.asarray(inputs[f"b{i}"], np.float32) for i in range(4)]
    wq = w[0] * gam[:, None]
    wk = w[1] * gam[:, None]
    wv = w[2] * gam[:, None]
    # per-key score bias must be ~zero for the fast path (true for this model)
    cq = wq.T @ bet + b[0]
    ck = wk.T @ bet + b[1]
    assert np.abs(cq).max() < 1e-30 and np.abs(ck).max() < 1e-30, \
        "nonzero q/k bias path not implemented"
    mt = (wk @ wq.T).T * WS          # lhsT layout: [cl, ci] = Mik^T
    w2s = wv * WS
    w3s = w[3] * 1.0
    cv = wv.T @ bet + b[2]
    b3p = b[3] + w[3].T @ cv

    def hilo(a):
        h = a.astype(E4)
        l_ = (a - h.astype(np.float32)).astype(E4)
        return h, l_

    mt_h, mt_l = hilo(mt)
    w2_h, w2_l = hilo(w2s)
    w3_h, w3_l = hilo(w3s)
    return dict(mt_h=mt_h, mt_l=mt_l, w2_h=w2_h, w2_l=w2_l,
                w3_h=w3_h, w3_l=w3_l, b3p=b3p.astype(np.float32))


def run(inputs, trace=False, nrep=1, **kw):
    nc = _get_nc(nrep)
    shared = _host_prep(inputs)
    x = np.ascontiguousarray(np.asarray(inputs["x"], dtype=np.float32))
    in_maps = [dict(shared, x=x[b]) for b in range(B)]
    res = run_bass_kernel_spmd(nc, in_maps, core_ids=list(range(B)), trace=trace, **kw)
    out = np.stack([res.results[b]["out"] for b in range(B)], axis=0)
    return out, res


def kernel(**inputs) -> np.ndarray:
    out, _ = run(inputs)
    return out


def make_bench_runner(inputs, nrep=1):
    """Reusable jitted shard_map callable + device-resident args."""
    import jax
    import concourse.mybir as _mybir
    from concourse import bass2jax as b2j
    from jax.experimental.shard_map import shard_map
    from jax.sharding import Mesh, PartitionSpec

    nc = _get_nc(nrep)
    b2j.install_neuronx_cc_hook()
    partition_name = nc.partition_id_tensor.name if nc.partition_id_tensor else None

    in_names, out_names, out_avals, zero_outs = [], [], [], []
    for alloc in nc.m.functions[0].allocations:
        if not isinstance(alloc, _mybir.MemoryLocationSet):
            continue
        name = alloc.memorylocations[0].name
        if alloc.kind == "ExternalInput":
            if name != partition_name:
                in_names.append(name)
        elif alloc.kind == "ExternalOutput":
            shape = tuple(alloc.tensor_shape)
            dtype = _mybir.dt.np(alloc.dtype)
            out_avals.append(jax.core.ShapedArray(shape, dtype))
            zero_outs.append(np.zeros(shape, dtype))
            out_names.append(name)
    n_params = len(in_names)
    all_names = in_names + out_names
    if partition_name is not None:
        all_names.append(partition_name)

    def _body(*args):
        operands = list(args)
        if partition_name is not None:
            operands.append(b2j.partition_id_tensor())
        outs = b2j._bass_exec_p.bind(
            *operands,
            out_avals=tuple(out_avals),
            in_names=tuple(all_names),
            out_names=tuple(out_names),
            lowering_input_output_aliases=(),
            sim_require_finite=True,
            sim_require_nnan=True,
            nc=nc,
        )
        return tuple(outs)

    shared = _host_prep(inputs)
    x = np.ascontiguousarray(np.asarray(inputs["x"], dtype=np.float32))
    in_maps = [dict(shared, x=x[b]) for b in range(B)]

    devices = jax.devices()[:B]
    mesh = Mesh(np.asarray(devices), ("core",))
    nin = n_params + len(out_names)
    sharded = jax.jit(
        shard_map(_body, mesh=mesh,
                  in_specs=(PartitionSpec("core"),) * nin,
                  out_specs=(PartitionSpec("core"),) * len(out_names),
                  check_rep=False),
        keep_unused=True,
    )
    concat_in = [np.concatenate([np.asarray(in_maps[c][nm]) for c in range(B)], axis=0)
                 for nm in in_names]
    concat_zeros = [np.zeros((B * z.shape[0], *z.shape[1:]), z.dtype) for z in zero_outs]
    args = [jax.device_put(a) for a in concat_in + concat_zeros]

    def call():
        return sharded(*args)

    return call, out_names, out_avals
